# revision 33
# baseline (speedup 1.0000x reference)
"""GQA attention block (B=2, L=2048, D=4096, H=32, HKV=8, RoPE, causal) on 8
Trainium2 NeuronCores.

Sharding: core c -> batch b=c//4, head-group g=c%4 (8 Q heads + 2 KV heads per
core).  Each core computes x[b] @ wq_g/wk_g projections, V^T directly (by
swapping matmul operands: stationary x-tile, moving wv^T -> no transposes),
RoPE, causal attention for its heads, and a partial output projection against
its slice of wo; the host sums the 4 partials per batch element.

All DMA'd operands are bf16 (half the HBM traffic of fp32; TRN2 matmul rate is
1 row/cycle for bf16 and fp32r alike, measured 224ns per 512-row matmul).
Scores are computed transposed S^T[j,l] in pairs of key tiles per [128,1024]
PSUM allocation so one exp activation covers two tiles.  The softmax
denominator stays off the PE's streaming path: DVE pair-sums accumulate
eacc[j,l] (f32r) per head and a single all-ones stationary matmul per (head,
l-chunk) folds the 128 key partitions (partition-broadcast denominator for
free); the fold is deferred two pipeline items so the PE never waits on the
DVE chain.  Causality: fully-masked key tiles are skipped; diagonal tiles are
zeroed post-exp with gpsimd affine_select (exp(s+m) == exp(s)*[m==0] for the
0/-1e9 mask).  The attention loop is software-pipelined flat across (head,
key-pair) items with LOOKAHEAD=3, and independent PE work is interleaved into
the Act-paced attention stretches: V^T of the next l-pair during attention of
the current one (PSUM drained via the scalar engine), the output projection
of the previous pair during the next pair's attention (weights DMA'd on the
sync queue and PSUM drained via DVE, keeping the scalar queue free for exp).
Startup races the first weight tiles (scalar queue) against the first x
chunks (sync + gpsimd queues).  Output partials are written bf16 (the host
upcasts and all-reduces).  Engine occupancy on hardware: PE ~95% at full
2.4GHz clock, ~690us of pure matmul streaming at the bf16 FLOP roofline.
"""

import numpy as np
import ml_dtypes

import concourse.mybir as mybir
import concourse.tile as tile
from concourse import bacc, bass_utils

B, L, D = 2, 2048, 4096
H, HKV, HD = 32, 8, 128
NCORES = 8
GROUPS = 4                # head groups (cores per batch element)
QH = H // GROUPS          # 8 q heads per core
KVH = HKV // GROUPS       # 2 kv heads per core
LC = 512                  # l-chunk
SCALE = 1.0 / float(np.sqrt(HD))
LOOKAHEAD = 3             # attention items (head, key-pair) in flight ahead of PV

f32 = mybir.dt.float32
f32r = mybir.dt.float32r
bf16 = mybir.dt.bfloat16
bfdt = ml_dtypes.bfloat16


def build_nc(seq_len=L):
    nlc = seq_len // LC            # 4
    npair = nlc // 2               # 2

    nc = bacc.Bacc(trn_type="TRN2")
    x_tl = nc.dram_tensor("x_tl", [npair * 2, 128, 32, 512], bf16, kind="ExternalInput")
    wqk_tl = nc.dram_tensor("wqk_tl", [KVH + QH, 128, 32 * 128], bf16, kind="ExternalInput")
    wvT_tl = nc.dram_tensor("wvT_tl", [128, 32, KVH * HD], bf16, kind="ExternalInput")
    wo_tl = nc.dram_tensor("wo_tl", [D // 128, 128, QH * 128], bf16, kind="ExternalInput")
    cosT = nc.dram_tensor("cosT", [64, seq_len], bf16, kind="ExternalInput")
    sinT = nc.dram_tensor("sinT", [64, seq_len], bf16, kind="ExternalInput")
    ones128 = nc.dram_tensor("ones128", [128, 128], f32, kind="ExternalInput")
    outT = nc.dram_tensor("outT", [D, seq_len], bf16, kind="ExternalOutput")

    with nc.allow_low_precision("bf16 attention kernel"), tile.TileContext(nc) as tc:
        with (
            tc.tile_pool(name="persist", bufs=1) as pp,
            tc.tile_pool(name="xp", bufs=2) as xp,
            tc.tile_pool(name="qp", bufs=1) as qp,
            tc.tile_pool(name="op", bufs=2) as op_,
            tc.tile_pool(name="wp", bufs=2) as wp,
            tc.tile_pool(name="wop", bufs=4) as wop,
            tc.tile_pool(name="ep", bufs=6) as ep,
            tc.tile_pool(name="tp", bufs=1) as tp,
            tc.tile_pool(name="mmps", bufs=2, space="PSUM") as mmps,
            tc.tile_pool(name="ops", bufs=2, space="PSUM") as ops_,
            tc.tile_pool(name="dps", bufs=1, space="PSUM") as dps,
            tc.tile_pool(name="vps_p", bufs=1, space="PSUM") as vps_p,
        ):
            # persistent SBUF tensors
            kT_p = {
                (kv, pi): pp.tile([128, 1024], bf16, tag=f"kT_{kv}_{pi}",
                                  name=f"kT_{kv}_{pi}")
                for kv in range(KVH) for pi in range(npair)
            }
            v_t = {
                lc: pp.tile([128, 4, KVH * HD], bf16, tag=f"v_{lc}", name=f"v_{lc}")
                for lc in range(nlc)
            }
            cs2 = pp.tile([128, seq_len], bf16)
            sn2 = pp.tile([128, seq_len], bf16)
            wvT = pp.tile([128, 32, KVH * HD], bf16)
            o128 = pp.tile([128, 128], f32r)

            # startup: first two weight tiles + first x chunks race on separate
            # queues; cos/sin etc. queue behind the weights on scalar.
            wts = {}
            for mi in range(2):
                wt = wp.tile([128, 32 * 128], bf16, tag="w", name=f"wt{mi}")
                nc.scalar.dma_start(wt[:, :2048], wqk_tl.ap()[mi][:, :2048])
                nc.scalar.dma_start(wt[:, 2048:], wqk_tl.ap()[mi][:, 2048:])
                wts[mi] = wt
            x_t = {}
            for lci in range(2):
                x_t[(0, lci)] = xp.tile([128, 32, 512], bf16, tag="x",
                                        name=f"x0_{lci}")
            for half in range(4):
                for lci in range(2):
                    eng = nc.sync if lci == 0 else nc.gpsimd
                    eng.dma_start(
                        x_t[(0, lci)][:, half * 8:(half + 1) * 8, :],
                        x_tl.ap()[lci][:, half * 8:(half + 1) * 8, :],
                    )
            nc.scalar.dma_start(cs2[0:64, :], cosT.ap())
            nc.scalar.dma_start(cs2[64:128, :], cosT.ap())
            nc.scalar.dma_start(sn2[0:64, :], sinT.ap())
            nc.scalar.dma_start(sn2[64:128, :], sinT.ap())
            # rotate-half form: out = t*cs2 + swap(t)*sn2 with sn2 = [-sin | sin]
            nc.vector.tensor_scalar_mul(sn2[0:64, :], sn2[0:64, :], -1.0)
            nc.scalar.dma_start(wvT[:], wvT_tl.ap())
            nc.scalar.dma_start(o128[:], ones128.ap().bitcast(f32r))

            # ---------------- emission generators ----------------

            def gen_projections(pi, q_pr):
                """K/Q projections + RoPE for pair pi; yields after each mi."""
                for mi in range(KVH + QH):      # k0,k1,q0..q7
                    if (pi, mi) == (0, 0) or (pi, mi) == (0, 1):
                        wt = wts[mi]
                    else:
                        wt = wp.tile([128, 32 * 128], bf16, tag="w",
                                     name=f"wt{pi}_{mi}")
                        nc.sync.dma_start(wt[:, :2048], wqk_tl.ap()[mi][:, :2048])
                        nc.sync.dma_start(wt[:, 2048:], wqk_tl.ap()[mi][:, 2048:])
                    ps2 = mmps.tile([128, 1024], f32, tag="big", name=f"ps{pi}_{mi}")
                    for dt in range(32):
                        for lci in range(2):
                            nc.tensor.matmul(
                                ps2[:, lci * 512:(lci + 1) * 512],
                                wt[:, dt * 128:(dt + 1) * 128],
                                x_t[(pi, lci)][:, dt, :],
                                start=(dt == 0), stop=(dt == 31),
                            )
                    cols = slice(pi * 1024, (pi + 1) * 1024)
                    t1 = tp.tile([128, 1024], f32, tag="t1", name=f"t1_{pi}_{mi}")
                    nc.vector.tensor_mul(t1[:], ps2[:], cs2[:, cols])
                    dst = kT_p[(mi, pi)][:] if mi < KVH else q_pr[:, mi - KVH, :]
                    nc.vector.tensor_mul(dst[0:64, :], ps2[64:128, :], sn2[0:64, cols])
                    nc.vector.tensor_mul(dst[64:128, :], ps2[0:64, :], sn2[64:128, cols])
                    nc.vector.tensor_tensor(dst, dst, t1[:], mybir.AluOpType.add)
                    yield

            def gen_vT(pi):
                """V^T for both l-chunks of pair pi; yields after each j-subtile."""
                for lci in range(2):
                    lc = 2 * pi + lci
                    for jj2 in range(2):
                        vps = vps_p.tile([128, 512], f32, tag="vps",
                                         name=f"vps{lc}_{jj2}")
                        for t in range(2):
                            jt = jj2 * 2 + t
                            for dt in range(32):
                                nc.tensor.matmul(
                                    vps[:, t * 256:(t + 1) * 256],
                                    x_t[(pi, lci)][:, dt, jt * 128:(jt + 1) * 128],
                                    wvT[:, dt, :],
                                    start=(dt == 0), stop=(dt == 31),
                                )
                            yield
                        nc.scalar.mul(
                            v_t[lc][:, jj2 * 2:(jj2 + 1) * 2, :],
                            vps[:].rearrange("p (a b) -> p a b", a=2), 1.0,
                        )

            def gen_attention(pi, q_pr, o2):
                """Causal attention for both l-chunks of pair pi, software-
                pipelined flat across (head, key-pair) items; yields per item."""
                for lci in range(2):
                    lc = 2 * pi + lci
                    njt = 4 * (lc + 1)
                    ngrp = njt // 2
                    items = [(h, g) for h in range(QH) for g in range(ngrp)]

                    def emit_scores(h, g, lc=lc, lci=lci):
                        kv = h // (QH // KVH)
                        psS2 = mmps.tile([128, 1024], f32, tag="big",
                                         name=f"psS{lc}_{h}_{g}")
                        for t in range(2):
                            jt = 2 * g + t
                            c0 = max(0, jt - 4 * lc) * 128
                            nc.tensor.matmul(
                                psS2[:, t * 512 + c0:(t + 1) * 512],
                                kT_p[(kv, jt // 8)][:, (jt % 8) * 128:(jt % 8 + 1) * 128],
                                q_pr[:, h, lci * 512 + c0:(lci + 1) * 512],
                                start=True, stop=True,
                            )
                        e = ep.tile([128, 1024], bf16, tag="e", name=f"e{lc}_{h}_{g}")
                        c00 = max(0, 2 * g - 4 * lc) * 128
                        c01 = max(0, 2 * g + 1 - 4 * lc) * 128
                        if c00 >= 256:
                            # steep diagonal group: exp only the live columns
                            nc.scalar.activation(
                                e[:, c00:512], psS2[:, c00:512],
                                mybir.ActivationFunctionType.Exp, scale=SCALE,
                            )
                            nc.scalar.activation(
                                e[:, 512 + c01:1024], psS2[:, 512 + c01:1024],
                                mybir.ActivationFunctionType.Exp, scale=SCALE,
                            )
                        else:
                            nc.scalar.activation(
                                e[:], psS2[:], mybir.ActivationFunctionType.Exp,
                                scale=SCALE,
                            )
                        for t in range(2):
                            jt = 2 * g + t
                            dg = jt - 4 * lc
                            if dg >= 0:
                                c0 = dg * 128
                                nc.gpsimd.affine_select(
                                    out=e[:, t * 512 + c0:t * 512 + c0 + 128],
                                    in_=e[:, t * 512 + c0:t * 512 + c0 + 128],
                                    compare_op=mybir.AluOpType.is_ge,
                                    fill=0.0,
                                    base=0,
                                    pattern=[[1, 128]],
                                    channel_multiplier=-1,
                                )
                        return e

                    po_t, eacc_t, ebuf, pending = {}, {}, {}, []
                    for idx in range(len(items) + LOOKAHEAD):
                        if idx < len(items):
                            ebuf[idx] = emit_scores(*items[idx])
                        j = idx - LOOKAHEAD
                        if j < 0:
                            yield
                            continue
                        h, g = items[j]
                        e = ebuf.pop(j)
                        if g == 0:
                            po_t[h] = ops_.tile([128, 512], f32, tag="po",
                                                name=f"po{lc}_{h}")
                            eacc_t[h] = tp.tile([128, 512], f32r, tag="eacc",
                                                bufs=2, name=f"eacc{lc}_{h}")
                        for t in range(2):
                            jt = 2 * g + t
                            c0 = max(0, jt - 4 * lc) * 128
                            nc.tensor.matmul(
                                po_t[h][:, c0:512],
                                v_t[jt // 4][:, jt % 4, (h // (QH // KVH)) * 128:
                                             (h // (QH // KVH) + 1) * 128],
                                e[:, t * 512 + c0:(t + 1) * 512],
                                start=(jt == 0), stop=(jt == njt - 1),
                                skip_group_check=True,
                            )
                        # denominator accumulation on DVE (keys stay on partitions)
                        c1 = max(0, 2 * g + 1 - 4 * lc) * 128
                        if g == 0 and c1 == 0:
                            nc.vector.tensor_tensor(
                                eacc_t[h][:], e[:, 0:512], e[:, 512:1024],
                                mybir.AluOpType.add,
                            )
                        elif g == 0:
                            nc.vector.tensor_copy(eacc_t[h][:], e[:, 0:512])
                            nc.vector.tensor_tensor(
                                eacc_t[h][:, c1:], eacc_t[h][:, c1:],
                                e[:, 512 + c1:1024], mybir.AluOpType.add,
                            )
                        elif c1 == 0:
                            pg = tp.tile([128, 512], bf16, tag="pg", bufs=1,
                                         name=f"pg{lc}_{h}_{g}")
                            nc.vector.tensor_tensor(
                                pg[:], e[:, 0:512], e[:, 512:1024],
                                mybir.AluOpType.add,
                            )
                            nc.vector.tensor_tensor(
                                eacc_t[h][:], eacc_t[h][:], pg[:],
                                mybir.AluOpType.add,
                            )
                        else:
                            for t in range(2):
                                c0 = max(0, 2 * g + t - 4 * lc) * 128
                                nc.vector.tensor_tensor(
                                    eacc_t[h][:, c0:], eacc_t[h][:, c0:],
                                    e[:, t * 512 + c0:(t + 1) * 512],
                                    mybir.AluOpType.add,
                                )
                        if g == ngrp - 1:
                            pending.append((h, j + 2))
                        while pending and pending[0][1] <= j:
                            # fold 128 key partitions with one ones-matmul,
                            # deferred so the PE never waits on the eacc chain
                            ph = pending.pop(0)[0]
                            pden = dps.tile([128, 512], f32, tag="pden",
                                            name=f"pden{lc}_{ph}")
                            nc.tensor.matmul(
                                pden[:], o128[:], eacc_t[ph][:],
                                start=True, stop=True,
                            )
                            rec = tp.tile([128, 512], f32, tag="rec", bufs=1,
                                          name=f"rec{lc}_{ph}")
                            nc.vector.reciprocal_approx_fast(out=rec[:], in_=pden[:])
                            nc.vector.tensor_mul(
                                o2[:, ph, lci * 512:(lci + 1) * 512], po_t[ph], rec[:]
                            )
                        yield
                    for ph, _ in pending:
                        pden = dps.tile([128, 512], f32, tag="pden",
                                        name=f"pdenx{lc}_{ph}")
                        nc.tensor.matmul(
                            pden[:], o128[:], eacc_t[ph][:], start=True, stop=True,
                        )
                        rec = tp.tile([128, 512], f32, tag="rec", bufs=1,
                                      name=f"recx{lc}_{ph}")
                        nc.vector.reciprocal_approx_fast(out=rec[:], in_=pden[:])
                        nc.vector.tensor_mul(
                            o2[:, ph, lci * 512:(lci + 1) * 512], po_t[ph], rec[:]
                        )
                    pending.clear()

            def gen_outproj(pi, o2, interleaved=False):
                """Partial output projection for pair pi; yields after each nt.
                The interleaved variant must not block the Act queue (exp lives
                there): weight DMAs go on sync, the PSUM drain on DVE."""
                for nt in range(D // 128):
                    wo_t = wop.tile([128, QH * 128], bf16, tag="wo",
                                    name=f"wo{pi}_{nt}")
                    (nc.sync if interleaved else nc.scalar).dma_start(
                        wo_t[:], wo_tl.ap()[nt])
                    pso2 = mmps.tile([128, 1024], f32, tag="big",
                                     name=f"pso{pi}_{nt}")
                    for h in range(QH):
                        for lci in range(2):
                            nc.tensor.matmul(
                                pso2[:, lci * 512:(lci + 1) * 512],
                                wo_t[:, h * 128:(h + 1) * 128],
                                o2[:, h, lci * 512:(lci + 1) * 512],
                                start=(h == 0), stop=(h == QH - 1),
                            )
                    ob = tp.tile([128, 1024], bf16, tag="ob", bufs=2,
                                 name=f"ob{pi}_{nt}")
                    if interleaved:
                        nc.vector.tensor_copy(ob[:], pso2[:])
                    else:
                        nc.scalar.mul(ob[:], pso2[:], 1.0)
                    nc.sync.dma_start(
                        outT.ap()[nt * 128:(nt + 1) * 128, pi * 1024:(pi + 1) * 1024],
                        ob[:],
                    )
                    yield

            def drain(gen):
                for _ in gen:
                    pass

            def interleave(main_gen, fill_gen, ratio):
                """Emit ratio items of main_gen per item of fill_gen; main first,
                then drain both."""
                n = 0
                for _ in main_gen:
                    n += 1
                    if n % ratio == 0:
                        next(fill_gen, None)
                drain(fill_gen)

            # ---------------- schedule ----------------
            q_pr0 = qp.tile([128, QH, 1024], bf16, tag="q", name="q_pr0")
            o2_0 = op_.tile([128, QH, 1024], bf16, tag="o2", name="o2_0")
            drain(gen_projections(0, q_pr0))
            drain(gen_vT(0))

            # pair-1 x DMA early so interleaved V^T(1) has data
            for lci in range(2):
                x_c = xp.tile([128, 32, 512], bf16, tag="x", name=f"x1_{lci}")
                for half in range(2):
                    nc.sync.dma_start(
                        x_c[:, half * 16:(half + 1) * 16, :],
                        x_tl.ap()[2 + lci][:, half * 16:(half + 1) * 16, :],
                    )
                x_t[(1, lci)] = x_c

            # attention(pair0) with V^T(pair1) filling Act-paced gaps
            interleave(gen_attention(0, q_pr0, o2_0), gen_vT(1), 6)

            q_pr1 = qp.tile([128, QH, 1024], bf16, tag="q", name="q_pr1")
            o2_1 = op_.tile([128, QH, 1024], bf16, tag="o2", name="o2_1")
            drain(gen_projections(1, q_pr1))

            # attention(pair1) with outproj(pair0) filling gaps
            interleave(gen_attention(1, q_pr1, o2_1), gen_outproj(0, o2_0, interleaved=True), 3)

            drain(gen_outproj(1, o2_1))
    nc.compile()
    return nc


_PERM = np.concatenate([np.arange(0, HD, 2), np.arange(1, HD, 2)])


def shard_inputs(x, wq, wk, wv, wo, cos, sin, mask, seq_len=L):
    """Build the 8 per-core input maps (host pre-tiling, bf16)."""
    nlc = seq_len // LC
    cosT = np.ascontiguousarray(np.asarray(cos)[:seq_len].T, dtype=bfdt)
    sinT = np.ascontiguousarray(np.asarray(sin)[:seq_len].T, dtype=bfdt)
    ones128 = np.ones((128, 128), np.float32)

    x_tls = []
    for b in range(B):
        xT = np.asarray(x[b, :seq_len]).T.astype(np.float32)   # [D, seq]
        # [lc, p, dt, c] = x[b, lc*512+c, dt*128+p]
        xv = xT.reshape(32, 128, nlc, 512).transpose(2, 1, 0, 3)
        x_tls.append(np.ascontiguousarray(xv, dtype=bfdt))

    def permute_rows(w):
        nh = w.shape[0] // HD
        wp_ = w.reshape(nh, HD, -1)[:, _PERM, :]
        return wp_.reshape(w.shape)

    in_maps = []
    for c in range(NCORES):
        b, g = divmod(c, GROUPS)
        wq_g = permute_rows(np.asarray(wq)[QH * HD * g:QH * HD * (g + 1)])
        wk_g = permute_rows(np.asarray(wk)[KVH * HD * g:KVH * HD * (g + 1)])
        wv_g = np.asarray(wv)[KVH * HD * g:KVH * HD * (g + 1)]
        wo_g = np.asarray(wo)[:, QH * HD * g:QH * HD * (g + 1)]
        # [mi, p, dt*128+mc] = W[mi*128+mc, dt*128+p]
        W = np.concatenate([wk_g, wq_g], axis=0)               # [1280, 4096]
        wqk_tl = np.ascontiguousarray(
            W.reshape(KVH + QH, 128, 32, 128).transpose(0, 3, 2, 1)
            .reshape(KVH + QH, 128, 32 * 128), dtype=bfdt)
        # [p, dt, d] = wv_g[d, dt*128+p]
        wvT_tl = np.ascontiguousarray(
            wv_g.reshape(KVH * HD, 32, 128).transpose(2, 1, 0), dtype=bfdt)
        # [nt, p, h*128+n] = wo_g[nt*128+n, h*128+p]
        wo_tl = np.ascontiguousarray(
            wo_g.reshape(D // 128, 128, QH, 128).transpose(0, 3, 2, 1)
            .reshape(D // 128, 128, QH * 128), dtype=bfdt)
        in_maps.append({
            "x_tl": x_tls[b],
            "wqk_tl": wqk_tl,
            "wvT_tl": wvT_tl,
            "wo_tl": wo_tl,
            "cosT": cosT,
            "sinT": sinT,
            "ones128": ones128,
        })
    return in_maps


def gather_output(results, seq_len=L):
    out = np.zeros((B, seq_len, D), np.float32)
    for c in range(NCORES):
        b = c // GROUPS
        out[b] += np.asarray(results[c]["outT"], dtype=np.float32).T
    return out


_nc_cache = {}


def _get_nc(seq_len=L):
    if seq_len not in _nc_cache:
        _nc_cache[seq_len] = build_nc(seq_len)
    return _nc_cache[seq_len]


def run_sharded(inputs, trace=False, tmpdir=None):
    nc = _get_nc()
    in_maps = shard_inputs(**inputs)
    res = bass_utils.run_bass_kernel_spmd(
        nc, in_maps, core_ids=list(range(NCORES)), trace=trace, tmpdir=tmpdir
    )
    return gather_output(res.results), res


def kernel(**inputs) -> np.ndarray:
    out, _ = run_sharded(inputs)
    return out
